# revision 27
# baseline (speedup 1.0000x reference)
"""Trainium2 Bass kernel for BinaryNN forward (binary conv net + log_softmax).

Contract: kernel(**inputs) takes FULL unsharded inputs
  x     [8192, 1, 28, 28] f32
  w1    [16, 1, 3, 3]     f32
  w2    [16, 16, 3, 3]    f32
  fc_w  [10, 2304]        f32
returns [8192, 10] f32 log_softmax logits.

Strategy: pure data parallel over 8 NeuronCores (batch 1024/core), conv lowered
to fp8 TensorEngine matmuls. v2 design from HW microbenchmarks:
  - conv1 (K=30) row-tiled 2x: two concurrent matmuls in PE row-groups 0/32
    (window data replicated at partition bases 0 and 32, pre-laid-out on host
    so the device DMA is a contiguous burst).
  - conv2: fp8 DoubleRow (dy0,dy1 as 2 K-planes in one pass, 2 planes/cycle)
    + single pass for dy2, per output row, N=512.
  - 2x2 avg-pool+sign: pool sums computed on PE as a DoubleRow matmul with a
    0/1 matrix (y-pair in the 2 planes, x-pair folded into the matrix),
    replacing the DVE add chain.
  - fc: chunk-pair DoubleRow matmuls (K=96 virtual) in an end-of-half burst so
    the logits PSUM bank borrows the conv2 pool's rotation.
  - every sign() is one PSUM->SBUF clamp/Sign instruction on [*,1024] tiles,
    alternating ACT and DVE to split the elementwise wall across both engines.
PSUM: conv1 pool 2x[128,1024] (4 banks) + conv2/pool/fc/transpose shared pool
2x[128,1024] (4 banks).
"""

import functools
import itertools as _it
import numpy as np
import ml_dtypes


def _chain(*gens):
    return _it.chain(*gens)

N_CORES = 8
B_TOTAL = 8192
B = B_TOTAL // N_CORES  # 1024 per core
BH = 512                # half-batch processed per outer iteration
THRESH = 0.2

FP8 = ml_dtypes.float8_e4m3


# ----------------------------------------------------------------------------
# Device program (built once, cached)
# ----------------------------------------------------------------------------

@functools.lru_cache(maxsize=1)
def _build_program():
    from contextlib import ExitStack
    import concourse.bass as bass
    import concourse.tile as tile
    import concourse.mybir as mybir
    from concourse import bacc

    f32 = mybir.dt.float32
    fp8 = mybir.dt.float8e4
    AF = mybir.ActivationFunctionType
    ALU = mybir.AluOpType
    AX = mybir.AxisListType
    DR = mybir.MatmulPerfMode.DoubleRow

    nc = bacc.Bacc(
        "TRN2",
        target_bir_lowering=False,
        debug=False,
        num_devices=N_CORES,
    )

    Y1 = 26          # conv1 out rows
    NW = 4           # x-windows
    WCOLS = Y1 * BH  # per-(w,h) window free size (13312)

    # host-prepacked, 2-replica conv1 window blocks: [8, 64, 26*512]
    xqr_t = nc.dram_tensor("xqr", [8, 128, WCOLS], fp8, kind="ExternalInput")
    wl1_t = nc.dram_tensor("wl1", [128, 128], fp8, kind="ExternalInput")
    wl2_t = nc.dram_tensor("wl2", [128, 384], fp8, kind="ExternalInput")
    wpl_t = nc.dram_tensor("wpl", [128, 96], fp8, kind="ExternalInput")
    wfc_t = nc.dram_tensor("wfc", [48, 768], fp8, kind="ExternalInput")
    idt_t = nc.dram_tensor("ident", [10, 10], f32, kind="ExternalInput")
    out_t = nc.dram_tensor("out", [B, 10], f32, kind="ExternalOutput")

    def emit(ctx, tc):
        wpool = ctx.enter_context(tc.tile_pool(name="weights", bufs=1))
        rhs1_pool = ctx.enter_context(tc.tile_pool(name="rhs1", bufs=2))
        a1_pool = ctx.enter_context(tc.tile_pool(name="a1", bufs=2))
        s2_pool = ctx.enter_context(tc.tile_pool(name="s2", bufs=2))
        psw_pool = ctx.enter_context(tc.tile_pool(name="psw", bufs=2))
        sm_pool = ctx.enter_context(tc.tile_pool(name="sm", bufs=10))
        ps_pool = ctx.enter_context(tc.tile_pool(name="ps", bufs=1, space="PSUM"))

        wl1 = wpool.tile([128, 128], fp8)
        nc.gpsimd.dma_start(wl1[:], wl1_t.ap())
        wl2 = wpool.tile([128, 384], fp8)
        nc.gpsimd.dma_start(wl2[:], wl2_t.ap())
        wpl = wpool.tile([128, 96], fp8)
        nc.gpsimd.dma_start(wpl[:], wpl_t.ap())
        wfc = wpool.tile([48, 768], fp8)
        nc.gpsimd.dma_start(wfc[:], wfc_t.ap())
        idt = wpool.tile([10, 10], f32)
        nc.gpsimd.dma_start(idt[:], idt_t.ap())
        lsb = wpool.tile([10, B], f32)  # logits staging, both halves

        eng = [0]

        def sign_to(dst, src):
            # src holds exact integers -> clamp(-1,1) == sign(); alternate
            # engines to split the PSUM->SBUF wall
            eng[0] ^= 1
            if eng[0]:
                nc.scalar.sign(dst, src)
            else:
                nc.vector.tensor_scalar(dst, src, -1.0, 1.0, ALU.max, ALU.min)

        def dma_rhs1(h, w):
            blk = h * NW + w
            rhs1 = rhs1_pool.tile([128, WCOLS], fp8, tag="rhs1", name="rhs1")
            for g in range(7):
                c0 = g * 2048
                cn = min(2048, WCOLS - c0)
                src = bass.AP(
                    xqr_t,
                    blk * 128 * WCOLS + c0,
                    [[WCOLS, 128], [1, cn]],
                )
                nc.sync.dma_start(rhs1[0:128, c0:c0 + cn], src)
            return rhs1

        def big_tile():
            return ps_pool.tile([128, 1024], f32, tag="big", name="bigt", bufs=3)

        def conv1_gen(rhs1, a1):
            # 7 row-tiled packs of 4 (last: 2), 4 concurrent row-groups
            for g in range(7):
                ny = min(4, Y1 - 4 * g)
                tiles = [big_tile() for _ in range((ny + 1) // 2)]
                for i in range(ny):
                    y = 4 * g + i
                    nc.tensor.matmul(
                        tiles[i // 2][:, (i % 2) * 512:(i % 2 + 1) * 512],
                        wl1[32 * i:32 * i + 30, :],
                        rhs1[32 * i:32 * i + 30, y * 512:(y + 1) * 512],
                        start=True, stop=True, tile_position=(32 * i, 0),
                    )
                    if i % 2 == 1:
                        # sign each tile as soon as its two matmuls are
                        # emitted, so the release enqueues earlier
                        j = i // 2
                        sign_to(a1[:, (4 * g + 2 * j) * 512:
                                   (4 * g + 2 * j + 2) * 512], tiles[j][:])
                yield

        def emit_pool(w, q, pool_srcs, pswh):
            # pool: DR matmul per py (y-pair = 2 planes, x-pair in matrix)
            psp = big_tile()
            for j, sc in enumerate(pool_srcs):
                nc.tensor.matmul(
                    psp[0:48, j * 512:(j + 1) * 512],
                    wpl[:].rearrange("p (two m) -> p two m", two=2),
                    sc.rearrange("p (two n) -> p two n", two=2),
                    start=True, stop=True, perf_mode=DR,
                )
            sign_to(pswh[0:48, (w * 6 + q) * 1024:(w * 6 + q + 1) * 1024],
                    psp[0:48, :])

        def conv2pool_gen(w, a1, s2, pswh):
            pend = None
            for q in range(6):        # py pairs
                pool_srcs = []
                for py in (2 * q, 2 * q + 1):
                    ps2 = big_tile()
                    for hy in range(2):
                        y = 2 * py + hy
                        nc.tensor.matmul(
                            ps2[:, hy * 512:(hy + 1) * 512],
                            wl2[:, 0:256].rearrange("p (two m) -> p two m", two=2),
                            a1[:, y * 512:(y + 2) * 512].rearrange(
                                "p (two n) -> p two n", two=2),
                            start=True, stop=False, perf_mode=DR,
                        )
                        nc.tensor.matmul(
                            ps2[:, hy * 512:(hy + 1) * 512],
                            wl2[:, 256:384],
                            a1[:, (y + 2) * 512:(y + 3) * 512],
                            start=False, stop=True,
                        )
                    sc = s2[:, py * 1024:(py + 1) * 1024]
                    sign_to(sc, ps2[:])
                    pool_srcs.append(sc)
                    # emit the delayed pool between this pair's two py's so
                    # the shared rotation's next reuse sits further from the
                    # sign that releases it
                    if py == 2 * q and pend is not None:
                        emit_pool(w, pend[0], pend[1], pswh)
                        pend = None
                pend = (q, pool_srcs)
                yield
            emit_pool(w, pend[0], pend[1], pswh)

        def fc_gen(lg, pswh, j0, j1):
            for j in range(j0, j1):
                nc.tensor.matmul(
                    lg,
                    wfc[:, j * 32:(j + 1) * 32].rearrange(
                        "p (two m) -> p two m", two=2),
                    pswh[0:48, j * 1024:(j + 1) * 1024].rearrange(
                        "p (two n) -> p two n", two=2),
                    start=(j == 0), stop=(j == 23), perf_mode=DR,
                )
                yield

        def softmax_gen(h, lg):
            nc.vector.tensor_copy(lsb[:, h * BH:(h + 1) * BH], lg[0:10, :])
            yield
            # log_softmax on 4 chunks of 128 images, ACT funcs grouped
            lqs, nms, ses = [], [], []
            for qq in range(4):
                q = 4 * h + qq
                if h == 1:
                    # standalone tail: big rotation is idle, use it so the
                    # 4 transposes pipeline instead of serializing on 1 bank
                    ptt = big_tile()
                else:
                    ptt = ps_pool.tile([128, 16], f32, tag="pt", name="ptt",
                                       bufs=1)
                nc.tensor.transpose(ptt[0:128, 0:10],
                                    lsb[:, q * 128:(q + 1) * 128], idt[:])
                lq = sm_pool.tile([128, 10], f32, tag=f"lq{qq}", name="lq")
                nc.vector.tensor_copy(lq[:], ptt[0:128, 0:10])
                nm = sm_pool.tile([128, 1], f32, tag=f"nm{qq}", name="nm")
                nc.vector.reduce_max(nm[:], lq[:], axis=AX.X, negate=True)
                lqs.append(lq)
                nms.append(nm)
                yield
            for qq in range(4):
                scr = sm_pool.tile([128, 10], f32, tag="scr", name="scr", bufs=2)
                se = sm_pool.tile([128, 1], f32, tag=f"se{qq}", name="se")
                nc.scalar.activation(scr[:], lqs[qq][:], AF.Exp,
                                     bias=nms[qq][:], accum_out=se[:])
                ses.append(se)
            yield
            lss = []
            for qq in range(4):
                ls = sm_pool.tile([128, 1], f32, tag=f"ls{qq}", name="ls")
                nc.scalar.activation(ls[:], ses[qq][:], AF.Ln)
                lss.append(ls)
            out_ap = out_t.ap()
            for qq in range(4):
                q = 4 * h + qq
                o = sm_pool.tile([128, 10], f32, tag="o", name="o", bufs=2)
                nc.vector.tensor_scalar(o[:], lqs[qq][:], nms[qq][:],
                                        lss[qq][:], ALU.add, ALU.subtract)
                nc.sync.dma_start(out_ap[q * 128:(q + 1) * 128, :], o[:])
                yield

        def drive(*pairs):
            active = [[g, wt] for g, wt in pairs if g is not None]
            while active:
                nxt = []
                for g, wt in active:
                    alive = True
                    for _ in range(wt):
                        try:
                            next(g)
                        except StopIteration:
                            alive = False
                            break
                    if alive:
                        nxt.append([g, wt])
                active = nxt

        tail = None
        nxt_half = [None]
        for h in range(2):
            pswh = psw_pool.tile([48, 24 * 1024], fp8, tag="pswh", name="pswh")
            lgt = ps_pool.tile([16, 512], f32, tag="lg", name="lgt", bufs=1)
            lg = lgt[0:16, 0:512]
            rhs1 = nxt_half[0] if h == 1 else dma_rhs1(h, 0)
            prev = None  # (w, a1, s2)
            for w in range(NW):
                a1 = a1_pool.tile([128, WCOLS], fp8, tag="a1", name="a1")
                c1 = conv1_gen(rhs1, a1)
                if w + 1 < NW:
                    rhs1 = dma_rhs1(h, w + 1)
                elif h == 0:
                    nxt_half[0] = dma_rhs1(1, 0)
                if prev is not None:
                    other = conv2pool_gen(prev[0], prev[1], prev[2], pswh)
                else:
                    other = tail  # fc tail + softmax of previous half
                drive((c1, 1), (other, 1))
                s2 = s2_pool.tile([128, 12 * 1024], fp8, tag="s2", name="s2")
                prev = (w, a1, s2)
            # last window's conv2/pool interleaved with fc of ready chunks
            drive((conv2pool_gen(prev[0], prev[1], prev[2], pswh), 1),
                  (fc_gen(lg, pswh, 0, 18), 3))
            tail = _chain(fc_gen(lg, pswh, 18, 24), softmax_gen(h, lg))
        drive((tail, 1))

    with tile.TileContext(nc) as tc:
        with ExitStack() as ctx:
            emit(ctx, tc)

    nc.compile()
    return nc


# ----------------------------------------------------------------------------
# Host-side packing
# ----------------------------------------------------------------------------

def _pack_weights(w1, w2, fc_w):
    w1s = np.sign(w1[:, 0].astype(np.float32))   # [16,3,3]
    w2s = np.sign(w2.astype(np.float32))         # [16,16,3,3]
    fcs = np.sign(fc_w.astype(np.float32))       # [10,2304]

    # conv1 Toeplitz: rows k=(dy,xi in 0..9), cols m=(o,xr in 0..7);
    # two replicas at partition bases 0 and 32 for row-tiling
    L1 = np.zeros((128, 128), np.float32)
    for o in range(16):
        for xr in range(8):
            for dy in range(3):
                for dx in range(3):
                    v = w1s[o, dy, dx]
                    for r in range(4):
                        L1[32 * r + dy * 10 + xr + dx, o * 8 + xr] = v

    # conv2 Toeplitz per dy: rows k=(c,xi in 0..7), cols j:
    #   j in [0,48):   o=j//3, xr=2*(j%3)      (even out-x)
    #   j in [64,112): o=(j-64)//3, xr=2*((j-64)%3)+1  (odd out-x)
    L2 = np.zeros((128, 384), np.float32)
    for dy in range(3):
        for c in range(16):
            for xi in range(8):
                k = c * 8 + xi
                for j in range(112):
                    if j < 48:
                        o, xr = j // 3, 2 * (j % 3)
                    elif j >= 64:
                        o, xr = (j - 64) // 3, 2 * ((j - 64) % 3) + 1
                    else:
                        continue
                    dx = xi - xr
                    if 0 <= dx <= 2:
                        if dy < 2:
                            L2[k, dy * 128 + j] = w2s[o, c, dy, dx]
                        else:
                            L2[k, 256 + j] = w2s[o, c, dy, dx]

    # pool matrix: out m=(o,pxl in 0..2) sums s2 partitions (even j, odd j);
    # DR: plane 0 and plane 1 identical (y-pair via rhs planes)
    P = np.zeros((128, 96), np.float32)
    for o in range(16):
        for pxl in range(3):
            m = o * 3 + pxl
            je = o * 3 + pxl          # even-x partition (j in [0,48))
            jo = 64 + o * 3 + pxl     # odd-x partition  (j in [64,112))
            for pl in range(2):
                P[je, pl * 48 + m] = 1.0
                P[jo, pl * 48 + m] = 1.0

    # fc chunk-pairs: pair j=(w*6+q) = chunks k0=(w,2q), k1=(w,2q+1),
    # k=(w,py): feature(p=(o,pxl)) = o*144 + py*12 + 3*w + pxl
    Lfc = np.zeros((48, 768), np.float32)
    for w in range(4):
        for q in range(6):
            j = w * 6 + q
            for pl in range(2):
                py = 2 * q + pl
                for p in range(48):
                    o, pxl = p // 3, p % 3
                    feat = o * 144 + py * 12 + 3 * w + pxl
                    Lfc[p, j * 32 + pl * 16:j * 32 + pl * 16 + 10] = fcs[:, feat]

    return (L1.astype(FP8), L2.astype(FP8), P.astype(FP8), Lfc.astype(FP8))


def _prep_inputs(x, w1, w2, fc_w):
    Y1 = 26
    xq = np.where(x.reshape(B_TOTAL, 28, 28) >= THRESH, 1.0, -1.0)
    xq_t = np.transpose(xq, (1, 2, 0)).astype(FP8)  # [28, 28, B_TOTAL]
    L1, L2, P, Lfc = _pack_weights(w1, w2, fc_w)
    ident = np.eye(10, dtype=np.float32)

    in_maps = []
    for i in range(N_CORES):
        xc = xq_t[:, :, i * B:(i + 1) * B]  # [28, 28, 1024]
        # window blocks: blk=(h,w): [64, 26*512] with taps (dy,xi) replicated
        # at partition bases 0 and 32; col (y,b) holds xq[y+dy, 6w+xi, h*512+b]
        xqr = np.zeros((8, 128, Y1 * BH), FP8)
        for h in range(2):
            for w in range(4):
                blk = h * 4 + w
                # [3dy, 10xi, 26y, 512b]
                base = np.stack([
                    np.stack([
                        xc[dy:dy + Y1, 6 * w + xi, h * BH:(h + 1) * BH]
                        for xi in range(10)
                    ], axis=0)
                    for dy in range(3)
                ], axis=0)
                flat = base.reshape(30, Y1 * BH)
                for r in range(4):
                    xqr[blk, 32 * r:32 * r + 30] = flat
        in_maps.append({
            "xqr": xqr, "wl1": L1, "wl2": L2, "wpl": P, "wfc": Lfc,
            "ident": ident,
        })
    return in_maps


# ----------------------------------------------------------------------------
# Entry point
# ----------------------------------------------------------------------------

TRACE = False
LAST_RESULTS = None


def kernel(x, w1, w2, fc_w):
    global LAST_RESULTS
    from concourse.bass_utils import run_bass_kernel_spmd

    x = np.asarray(x)
    in_maps = _prep_inputs(x, np.asarray(w1), np.asarray(w2), np.asarray(fc_w))
    nc = _build_program()
    res = run_bass_kernel_spmd(nc, in_maps, list(range(N_CORES)), trace=TRACE)
    LAST_RESULTS = res
    out = np.concatenate(
        [np.asarray(res.results[i]["out"]) for i in range(N_CORES)], axis=0
    )
    return out.astype(np.float32)


# revision 28
# speedup vs baseline: 1.0028x; 1.0028x over previous
"""Trainium2 Bass kernel for BinaryNN forward (binary conv net + log_softmax).

Contract: kernel(**inputs) takes FULL unsharded inputs
  x     [8192, 1, 28, 28] f32
  w1    [16, 1, 3, 3]     f32
  w2    [16, 16, 3, 3]    f32
  fc_w  [10, 2304]        f32
returns [8192, 10] f32 log_softmax logits.

Strategy: pure data parallel over 8 NeuronCores (batch 1024/core), conv lowered
to fp8 TensorEngine matmuls. v2 design from HW microbenchmarks:
  - conv1 (K=30) row-tiled 2x: two concurrent matmuls in PE row-groups 0/32
    (window data replicated at partition bases 0 and 32, pre-laid-out on host
    so the device DMA is a contiguous burst).
  - conv2: fp8 DoubleRow (dy0,dy1 as 2 K-planes in one pass, 2 planes/cycle)
    + single pass for dy2, per output row, N=512.
  - 2x2 avg-pool+sign: pool sums computed on PE as a DoubleRow matmul with a
    0/1 matrix (y-pair in the 2 planes, x-pair folded into the matrix),
    replacing the DVE add chain.
  - fc: chunk-pair DoubleRow matmuls (K=96 virtual) in an end-of-half burst so
    the logits PSUM bank borrows the conv2 pool's rotation.
  - every sign() is one PSUM->SBUF clamp/Sign instruction on [*,1024] tiles,
    alternating ACT and DVE to split the elementwise wall across both engines.
PSUM: conv1 pool 2x[128,1024] (4 banks) + conv2/pool/fc/transpose shared pool
2x[128,1024] (4 banks).
"""

import functools
import itertools as _it
import numpy as np
import ml_dtypes


def _chain(*gens):
    return _it.chain(*gens)

N_CORES = 8
B_TOTAL = 8192
B = B_TOTAL // N_CORES  # 1024 per core
BH = 512                # half-batch processed per outer iteration
THRESH = 0.2

FP8 = ml_dtypes.float8_e4m3


# ----------------------------------------------------------------------------
# Device program (built once, cached)
# ----------------------------------------------------------------------------

@functools.lru_cache(maxsize=1)
def _build_program():
    from contextlib import ExitStack
    import concourse.bass as bass
    import concourse.tile as tile
    import concourse.mybir as mybir
    from concourse import bacc

    f32 = mybir.dt.float32
    fp8 = mybir.dt.float8e4
    AF = mybir.ActivationFunctionType
    ALU = mybir.AluOpType
    AX = mybir.AxisListType
    DR = mybir.MatmulPerfMode.DoubleRow

    nc = bacc.Bacc(
        "TRN2",
        target_bir_lowering=False,
        debug=False,
        num_devices=N_CORES,
    )

    Y1 = 26          # conv1 out rows
    NW = 4           # x-windows
    WCOLS = Y1 * BH  # per-(w,h) window free size (13312)

    # host-prepacked, 2-replica conv1 window blocks: [8, 64, 26*512]
    xqr_t = nc.dram_tensor("xqr", [8, 128, WCOLS], fp8, kind="ExternalInput")
    wl1_t = nc.dram_tensor("wl1", [128, 128], fp8, kind="ExternalInput")
    wl2_t = nc.dram_tensor("wl2", [128, 384], fp8, kind="ExternalInput")
    wpl_t = nc.dram_tensor("wpl", [128, 96], fp8, kind="ExternalInput")
    wfc_t = nc.dram_tensor("wfc", [48, 768], fp8, kind="ExternalInput")
    idt_t = nc.dram_tensor("ident", [10, 10], f32, kind="ExternalInput")
    out_t = nc.dram_tensor("out", [B, 10], f32, kind="ExternalOutput")

    def emit(ctx, tc):
        wpool = ctx.enter_context(tc.tile_pool(name="weights", bufs=1))
        rhs1_pool = ctx.enter_context(tc.tile_pool(name="rhs1", bufs=2))
        a1_pool = ctx.enter_context(tc.tile_pool(name="a1", bufs=2))
        s2_pool = ctx.enter_context(tc.tile_pool(name="s2", bufs=2))
        psw_pool = ctx.enter_context(tc.tile_pool(name="psw", bufs=2))
        sm_pool = ctx.enter_context(tc.tile_pool(name="sm", bufs=10))
        ps_pool = ctx.enter_context(tc.tile_pool(name="ps", bufs=1, space="PSUM"))

        wl1 = wpool.tile([128, 128], fp8)
        nc.gpsimd.dma_start(wl1[:], wl1_t.ap())
        wl2 = wpool.tile([128, 384], fp8)
        nc.gpsimd.dma_start(wl2[:], wl2_t.ap())
        wpl = wpool.tile([128, 96], fp8)
        nc.gpsimd.dma_start(wpl[:], wpl_t.ap())
        wfc = wpool.tile([48, 768], fp8)
        nc.gpsimd.dma_start(wfc[:], wfc_t.ap())
        idt = wpool.tile([10, 10], f32)
        nc.gpsimd.dma_start(idt[:], idt_t.ap())
        lsb = wpool.tile([10, B], f32)  # logits staging, both halves

        eng = [0]

        def sign_to(dst, src):
            # src holds exact integers -> clamp(-1,1) == sign(); alternate
            # engines to split the PSUM->SBUF wall
            eng[0] ^= 1
            if eng[0]:
                nc.scalar.sign(dst, src)
            else:
                nc.vector.tensor_scalar(dst, src, -1.0, 1.0, ALU.max, ALU.min)

        def dma_rhs1(h, w):
            blk = h * NW + w
            rhs1 = rhs1_pool.tile([128, WCOLS], fp8, tag="rhs1", name="rhs1")
            for g in range(7):
                c0 = g * 2048
                cn = min(2048, WCOLS - c0)
                src = bass.AP(
                    xqr_t,
                    blk * 128 * WCOLS + c0,
                    [[WCOLS, 128], [1, cn]],
                )
                nc.sync.dma_start(rhs1[0:128, c0:c0 + cn], src)
            return rhs1

        def big_tile():
            return ps_pool.tile([128, 1024], f32, tag="big", name="bigt", bufs=3)

        def conv1_gen(rhs1, a1):
            # 7 row-tiled packs of 4 (last: 2), 4 concurrent row-groups
            for g in range(7):
                ny = min(4, Y1 - 4 * g)
                tiles = [big_tile() for _ in range((ny + 1) // 2)]
                for i in range(ny):
                    y = 4 * g + i
                    nc.tensor.matmul(
                        tiles[i // 2][:, (i % 2) * 512:(i % 2 + 1) * 512],
                        wl1[32 * i:32 * i + 30, :],
                        rhs1[32 * i:32 * i + 30, y * 512:(y + 1) * 512],
                        start=True, stop=True, tile_position=(32 * i, 0),
                    )
                    if i % 2 == 1:
                        # sign each tile as soon as its two matmuls are
                        # emitted, so the release enqueues earlier
                        j = i // 2
                        sign_to(a1[:, (4 * g + 2 * j) * 512:
                                   (4 * g + 2 * j + 2) * 512], tiles[j][:])
                yield

        def emit_pool(w, q, pool_srcs, pswh):
            # pool: DR matmul per py (y-pair = 2 planes, x-pair in matrix)
            psp = big_tile()
            for j, sc in enumerate(pool_srcs):
                nc.tensor.matmul(
                    psp[0:48, j * 512:(j + 1) * 512],
                    wpl[:].rearrange("p (two m) -> p two m", two=2),
                    sc.rearrange("p (two n) -> p two n", two=2),
                    start=True, stop=True, perf_mode=DR,
                )
            sign_to(pswh[0:48, (w * 6 + q) * 1024:(w * 6 + q + 1) * 1024],
                    psp[0:48, :])

        def conv2pool_gen(w, a1, s2, pswh):
            pend = None
            for q in range(6):        # py pairs
                pool_srcs = []
                for py in (2 * q, 2 * q + 1):
                    ps2 = big_tile()
                    for hy in range(2):
                        y = 2 * py + hy
                        nc.tensor.matmul(
                            ps2[:, hy * 512:(hy + 1) * 512],
                            wl2[:, 0:256].rearrange("p (two m) -> p two m", two=2),
                            a1[:, y * 512:(y + 2) * 512].rearrange(
                                "p (two n) -> p two n", two=2),
                            start=True, stop=False, perf_mode=DR,
                        )
                        nc.tensor.matmul(
                            ps2[:, hy * 512:(hy + 1) * 512],
                            wl2[:, 256:384],
                            a1[:, (y + 2) * 512:(y + 3) * 512],
                            start=False, stop=True,
                        )
                    sc = s2[:, py * 1024:(py + 1) * 1024]
                    sign_to(sc, ps2[:])
                    pool_srcs.append(sc)
                    # emit the delayed pool between this pair's two py's so
                    # the shared rotation's next reuse sits further from the
                    # sign that releases it
                    if py == 2 * q and pend is not None:
                        emit_pool(w, pend[0], pend[1], pswh)
                        pend = None
                pend = (q, pool_srcs)
                yield
            emit_pool(w, pend[0], pend[1], pswh)

        def fc_gen(lg, pswh, j0, j1):
            for j in range(j0, j1):
                nc.tensor.matmul(
                    lg,
                    wfc[:, j * 32:(j + 1) * 32].rearrange(
                        "p (two m) -> p two m", two=2),
                    pswh[0:48, j * 1024:(j + 1) * 1024].rearrange(
                        "p (two n) -> p two n", two=2),
                    start=(j == 0), stop=(j == 23), perf_mode=DR,
                )
                yield

        def softmax_gen(h, lg):
            nc.vector.tensor_copy(lsb[:, h * BH:(h + 1) * BH], lg[0:10, :])
            yield
            # log_softmax on 4 chunks of 128 images, ACT funcs grouped
            lqs, nms, ses = [], [], []
            for qq in range(4):
                q = 4 * h + qq
                ptt = ps_pool.tile([128, 16], f32, tag="pt", name="ptt", bufs=1)
                nc.tensor.transpose(ptt[0:128, 0:10],
                                    lsb[:, q * 128:(q + 1) * 128], idt[:])
                lq = sm_pool.tile([128, 10], f32, tag=f"lq{qq}", name="lq")
                nc.vector.tensor_copy(lq[:], ptt[0:128, 0:10])
                nm = sm_pool.tile([128, 1], f32, tag=f"nm{qq}", name="nm")
                nc.vector.reduce_max(nm[:], lq[:], axis=AX.X, negate=True)
                lqs.append(lq)
                nms.append(nm)
                yield
            for qq in range(4):
                scr = sm_pool.tile([128, 10], f32, tag="scr", name="scr", bufs=2)
                se = sm_pool.tile([128, 1], f32, tag=f"se{qq}", name="se")
                nc.scalar.activation(scr[:], lqs[qq][:], AF.Exp,
                                     bias=nms[qq][:], accum_out=se[:])
                ses.append(se)
            yield
            lss = []
            for qq in range(4):
                ls = sm_pool.tile([128, 1], f32, tag=f"ls{qq}", name="ls")
                nc.scalar.activation(ls[:], ses[qq][:], AF.Ln)
                lss.append(ls)
            out_ap = out_t.ap()
            for qq in range(4):
                q = 4 * h + qq
                o = sm_pool.tile([128, 10], f32, tag="o", name="o", bufs=2)
                nc.vector.tensor_scalar(o[:], lqs[qq][:], nms[qq][:],
                                        lss[qq][:], ALU.add, ALU.subtract)
                nc.sync.dma_start(out_ap[q * 128:(q + 1) * 128, :], o[:])
                yield

        def drive(*pairs):
            active = [[g, wt] for g, wt in pairs if g is not None]
            while active:
                nxt = []
                for g, wt in active:
                    alive = True
                    for _ in range(wt):
                        try:
                            next(g)
                        except StopIteration:
                            alive = False
                            break
                    if alive:
                        nxt.append([g, wt])
                active = nxt

        tail = None
        for h in range(2):
            pswh = psw_pool.tile([48, 24 * 1024], fp8, tag="pswh", name="pswh")
            lgt = ps_pool.tile([16, 512], f32, tag="lg", name="lgt", bufs=1)
            lg = lgt[0:16, 0:512]
            rhs1 = dma_rhs1(h, 0)
            prev = None  # (w, a1, s2)
            for w in range(NW):
                a1 = a1_pool.tile([128, WCOLS], fp8, tag="a1", name="a1")
                c1 = conv1_gen(rhs1, a1)
                if w + 1 < NW:
                    rhs1 = dma_rhs1(h, w + 1)
                if prev is not None:
                    other = conv2pool_gen(prev[0], prev[1], prev[2], pswh)
                else:
                    other = tail  # fc tail + softmax of previous half
                drive((c1, 1), (other, 1))
                s2 = s2_pool.tile([128, 12 * 1024], fp8, tag="s2", name="s2")
                prev = (w, a1, s2)
            # last window's conv2/pool interleaved with fc of ready chunks
            drive((conv2pool_gen(prev[0], prev[1], prev[2], pswh), 1),
                  (fc_gen(lg, pswh, 0, 18), 3))
            tail = _chain(fc_gen(lg, pswh, 18, 24), softmax_gen(h, lg))
        drive((tail, 1))

    with tile.TileContext(nc) as tc:
        with ExitStack() as ctx:
            emit(ctx, tc)

    nc.compile()
    return nc


# ----------------------------------------------------------------------------
# Host-side packing
# ----------------------------------------------------------------------------

def _pack_weights(w1, w2, fc_w):
    w1s = np.sign(w1[:, 0].astype(np.float32))   # [16,3,3]
    w2s = np.sign(w2.astype(np.float32))         # [16,16,3,3]
    fcs = np.sign(fc_w.astype(np.float32))       # [10,2304]

    # conv1 Toeplitz: rows k=(dy,xi in 0..9), cols m=(o,xr in 0..7);
    # two replicas at partition bases 0 and 32 for row-tiling
    L1 = np.zeros((128, 128), np.float32)
    for o in range(16):
        for xr in range(8):
            for dy in range(3):
                for dx in range(3):
                    v = w1s[o, dy, dx]
                    for r in range(4):
                        L1[32 * r + dy * 10 + xr + dx, o * 8 + xr] = v

    # conv2 Toeplitz per dy: rows k=(c,xi in 0..7), cols j:
    #   j in [0,48):   o=j//3, xr=2*(j%3)      (even out-x)
    #   j in [64,112): o=(j-64)//3, xr=2*((j-64)%3)+1  (odd out-x)
    L2 = np.zeros((128, 384), np.float32)
    for dy in range(3):
        for c in range(16):
            for xi in range(8):
                k = c * 8 + xi
                for j in range(112):
                    if j < 48:
                        o, xr = j // 3, 2 * (j % 3)
                    elif j >= 64:
                        o, xr = (j - 64) // 3, 2 * ((j - 64) % 3) + 1
                    else:
                        continue
                    dx = xi - xr
                    if 0 <= dx <= 2:
                        if dy < 2:
                            L2[k, dy * 128 + j] = w2s[o, c, dy, dx]
                        else:
                            L2[k, 256 + j] = w2s[o, c, dy, dx]

    # pool matrix: out m=(o,pxl in 0..2) sums s2 partitions (even j, odd j);
    # DR: plane 0 and plane 1 identical (y-pair via rhs planes)
    P = np.zeros((128, 96), np.float32)
    for o in range(16):
        for pxl in range(3):
            m = o * 3 + pxl
            je = o * 3 + pxl          # even-x partition (j in [0,48))
            jo = 64 + o * 3 + pxl     # odd-x partition  (j in [64,112))
            for pl in range(2):
                P[je, pl * 48 + m] = 1.0
                P[jo, pl * 48 + m] = 1.0

    # fc chunk-pairs: pair j=(w*6+q) = chunks k0=(w,2q), k1=(w,2q+1),
    # k=(w,py): feature(p=(o,pxl)) = o*144 + py*12 + 3*w + pxl
    Lfc = np.zeros((48, 768), np.float32)
    for w in range(4):
        for q in range(6):
            j = w * 6 + q
            for pl in range(2):
                py = 2 * q + pl
                for p in range(48):
                    o, pxl = p // 3, p % 3
                    feat = o * 144 + py * 12 + 3 * w + pxl
                    Lfc[p, j * 32 + pl * 16:j * 32 + pl * 16 + 10] = fcs[:, feat]

    return (L1.astype(FP8), L2.astype(FP8), P.astype(FP8), Lfc.astype(FP8))


def _prep_inputs(x, w1, w2, fc_w):
    Y1 = 26
    xq = np.where(x.reshape(B_TOTAL, 28, 28) >= THRESH, 1.0, -1.0)
    xq_t = np.transpose(xq, (1, 2, 0)).astype(FP8)  # [28, 28, B_TOTAL]
    L1, L2, P, Lfc = _pack_weights(w1, w2, fc_w)
    ident = np.eye(10, dtype=np.float32)

    in_maps = []
    for i in range(N_CORES):
        xc = xq_t[:, :, i * B:(i + 1) * B]  # [28, 28, 1024]
        # window blocks: blk=(h,w): [64, 26*512] with taps (dy,xi) replicated
        # at partition bases 0 and 32; col (y,b) holds xq[y+dy, 6w+xi, h*512+b]
        xqr = np.zeros((8, 128, Y1 * BH), FP8)
        for h in range(2):
            for w in range(4):
                blk = h * 4 + w
                # [3dy, 10xi, 26y, 512b]
                base = np.stack([
                    np.stack([
                        xc[dy:dy + Y1, 6 * w + xi, h * BH:(h + 1) * BH]
                        for xi in range(10)
                    ], axis=0)
                    for dy in range(3)
                ], axis=0)
                flat = base.reshape(30, Y1 * BH)
                for r in range(4):
                    xqr[blk, 32 * r:32 * r + 30] = flat
        in_maps.append({
            "xqr": xqr, "wl1": L1, "wl2": L2, "wpl": P, "wfc": Lfc,
            "ident": ident,
        })
    return in_maps


# ----------------------------------------------------------------------------
# Entry point
# ----------------------------------------------------------------------------

TRACE = False
LAST_RESULTS = None


def kernel(x, w1, w2, fc_w):
    global LAST_RESULTS
    from concourse.bass_utils import run_bass_kernel_spmd

    x = np.asarray(x)
    in_maps = _prep_inputs(x, np.asarray(w1), np.asarray(w2), np.asarray(fc_w))
    nc = _build_program()
    res = run_bass_kernel_spmd(nc, in_maps, list(range(N_CORES)), trace=TRACE)
    LAST_RESULTS = res
    out = np.concatenate(
        [np.asarray(res.results[i]["out"]) for i in range(N_CORES)], axis=0
    )
    return out.astype(np.float32)
